# revision 3
# baseline (speedup 1.0000x reference)
"""DynamicEdgeConvNet on Trainium (8 NeuronCores via jax on the axon backend).

Strategy ladder (first one that works wins):
  1. SPMD data-parallel jax.pmap over the 8 local devices: the O(N^2)
     dynamic-kNN + top-k and the edge MLP (dominant FLOPs) are sharded by
     node block; segment-sum partials are combined with an on-device psum;
     node-update MLP + layernorm run replicated so `nodes` stays identical
     on every core (no gathers needed).
  2. Whole-model jit on a single device.
  3. Eager jax (always works).

Hardcoded problem shape: N=16384 nodes, K=20, G=4, LATENT=HIDDEN=128,
2 hidden MLP layers, 3 message-passing steps, 3 in/out features.
"""

import numpy as np
import jax
import jax.numpy as jnp
from functools import partial

N = 16384
K = 20
G = 4
LATENT = 128
HIDDEN = 128
STEPS = 3
IN_FEATURES = 3
LN_EPS = 1e-6
NCORES = 8
NLOC = N // NCORES
ELOC = NLOC * K


def _mlp(x, layers):
    n = len(layers)
    for i, (W, b) in enumerate(layers):
        x = x @ W + b
        if i < n - 1:
            x = jax.nn.gelu(x)
    return x


def _layernorm(x, scale, bias):
    mu = jnp.mean(x, axis=-1, keepdims=True)
    var = jnp.mean(jnp.square(x - mu), axis=-1, keepdims=True)
    return (x - mu) * jax.lax.rsqrt(var + LN_EPS) * scale + bias


def _model(x, gv, senders, receivers, params, *, core_id=None):
    """Reference-equivalent model. If core_id is not None, runs the SPMD
    shard (inside pmap, axis name "i"); otherwise runs the full model."""
    spmd = core_id is not None
    gv = gv.reshape(1, -1)
    nodes = _mlp(x, params["embed"])

    if spmd:
        row0 = core_id * NLOC
        e0 = core_id * ELOC
        senders_loc = jax.lax.dynamic_slice(senders, (e0,), (ELOC,))
        receivers_loc = jax.lax.dynamic_slice(receivers, (e0,), (ELOC,))
        n_edges = ELOC
    else:
        senders_loc, receivers_loc = senders, receivers
        n_edges = senders.shape[0]

    edges = None
    for step in range(STEPS):
        gE = jnp.broadcast_to(gv, (n_edges, G))
        gN = jnp.broadcast_to(gv, (N, G))
        diff = nodes[senders_loc] - nodes[receivers_loc]
        if edges is None:
            einp = jnp.concatenate([diff, gE], axis=1)
        else:
            einp = jnp.concatenate([edges, diff, gE], axis=1)
        edges = _mlp(einp, params["edge"][step])

        recv = jax.ops.segment_sum(edges, receivers_loc, num_segments=N)
        if spmd:
            recv = jax.lax.psum(recv, axis_name="i")

        new_nodes = _mlp(jnp.concatenate([nodes, recv, gN], axis=1),
                         params["node"][step])
        nodes = nodes + new_nodes
        s_, b_ = params["ln_scale"][step], params["ln_bias"][step]
        nodes = _layernorm(nodes, s_, b_)
        edges = _layernorm(edges, s_, b_)

        if step != STEPS - 1:
            sq = jnp.sum(nodes * nodes, axis=1)
            if spmd:
                rows = jax.lax.dynamic_slice(nodes, (row0, 0), (NLOC, LATENT))
                sq_r = jax.lax.dynamic_slice(sq, (row0,), (NLOC,))
                d = sq_r[:, None] + sq[None, :] - 2.0 * (rows @ nodes.T)
                _, idx = jax.lax.top_k(-d, K)
                receivers_loc = idx.reshape(-1).astype(jnp.int32)
                senders_loc = jnp.repeat(
                    jnp.arange(NLOC, dtype=jnp.int32) + row0, K)
            else:
                d = sq[:, None] + sq[None, :] - 2.0 * (nodes @ nodes.T)
                _, idx = jax.lax.top_k(-d, K)
                receivers_loc = idx.reshape(-1).astype(jnp.int32)
                senders_loc = jnp.repeat(jnp.arange(N, dtype=jnp.int32), K)

    if spmd:
        out_loc = _mlp(
            jax.lax.dynamic_slice(nodes, (row0, 0), (NLOC, LATENT)),
            params["decoder"])
        out = jax.lax.all_gather(out_loc, axis_name="i", axis=0)
        return out.reshape(N, IN_FEATURES)
    return _mlp(nodes, params["decoder"])


@partial(jax.pmap, axis_name="i")
def _run_spmd(core_id, x, g, senders, receivers, params):
    return _model(x, g, senders, receivers, params, core_id=core_id[0])


@jax.jit
def _run_single(x, g, senders, receivers, params):
    return _model(x, g, senders, receivers, params)


def kernel(x, globals_, senders, receivers, params):
    x = np.asarray(x, dtype=np.float32)
    g = np.asarray(globals_, dtype=np.float32)
    senders = np.asarray(senders, dtype=np.int32)
    receivers = np.asarray(receivers, dtype=np.int32)

    # Eager per-op execution: on the neuron/axon backend, whole-model
    # jit/pmap graphs of this size hit internal compiler errors that also
    # kill the device connection, so per-op dispatch is the reliable path
    # (ops compile individually and cache).
    out = _model(jnp.asarray(x), jnp.asarray(g), jnp.asarray(senders),
                 jnp.asarray(receivers), jax.tree.map(jnp.asarray, params))
    return np.asarray(out)
